# revision 17
# baseline (speedup 1.0000x reference)
"""Trainium2 Bass kernel for nn_Neuron_83889301226253.

Computation (B=1024, D=32768, fp32):
    fatigue[b]   = 0.9 ** b
    mask         = (release_u < 0.9)
    ws[b]        = fatigue[b] * sum_d mask[b,d] * w[d] * x[b,d]
    noisy_thr[b] = thr[0] + noise_eps[b] * 1e-5
    out[b]       = tanh(ws[b]) if ws[b] > noisy_thr[b] else 0

Sharding: data-parallel over batch across 8 NeuronCores (128 rows each).
w/thr replicated; fatigue passed per-shard (function of global batch index).

Per-core dataflow: w is broadcast once to all 128 partitions (log2 doubling
SBUF->SBUF DMAs), then for each D-chunk the kernel streams x and release_u
tiles [128 x CHUNK] and does exactly two VectorE ops per chunk:
  1. xw = x * w_bcast                      (tensor_tensor mult)
  2. STT: (u < 0.9) * xw, fused accum_out  (mask+mul+row-sum in one op)
The per-row epilogue (fatigue scale, noisy threshold, compare, tanh, gate)
runs on [128,1] tiles and is negligible.
"""

import sys

import numpy as np

if "/opt/trn_rl_repo" not in sys.path:
    sys.path.insert(0, "/opt/trn_rl_repo")

B, D = 1024, 32768
NCORES = 8
BS = B // NCORES  # 128 rows per core == SBUF partition count
RELEASE_P = 0.9
FATIGUE_DECAY = 0.9
NOISE_SCALE = 1e-5
CHUNK = 4096
# ramp-down only: big chunks maximize DMA descriptor size (VectorE has
# slack, so a late start is free); small tail chunks shorten the compute
# left after the last byte lands
CHUNK_SIZES = [CHUNK] * 7 + [2048, 1024, 1024]
assert sum(CHUNK_SIZES) == D
NCHUNK = len(CHUNK_SIZES)
MMN = 512          # matmul moving-dim limit (one PSUM bank)
PSUM_TILE = 2048   # one PSUM wb tile (4 banks)

_NC_CACHE = None


def _build():
    import concourse.bacc as bacc
    import concourse.mybir as mybir
    from concourse.tile import TileContext

    f32 = mybir.dt.float32
    bf16 = mybir.dt.bfloat16
    P = BS
    nc = bacc.Bacc(None)
    x_d = nc.dram_tensor("x", [P, D], f32, kind="ExternalInput")
    u_d = nc.dram_tensor("u", [P, D], f32, kind="ExternalInput")
    w3_d = nc.dram_tensor("w3", [3, D], bf16, kind="ExternalInput")
    fat_d = nc.dram_tensor("fatigue", [P], f32, kind="ExternalInput")
    eps_d = nc.dram_tensor("eps", [P], f32, kind="ExternalInput")
    thr_d = nc.dram_tensor("thr", [1], f32, kind="ExternalInput")
    out_d = nc.dram_tensor("out", [P], f32, kind="ExternalOutput")

    with TileContext(nc) as tc:
        with tc.tile_pool(name="work", bufs=5) as pool, \
             tc.tile_pool(name="wrow", bufs=3) as wpool, \
             tc.tile_pool(name="psum", bufs=2, space="PSUM") as ppool, \
             tc.tile_pool(name="small", bufs=1) as spool:
            ones = spool.tile([3, P], bf16)
            nc.gpsimd.memset(ones[:], 1.0)

            # tiny epilogue inputs: SWDGE ring (idle) so they land early
            fat = spool.tile([P, 1], f32)
            nc.gpsimd.dma_start(out=fat[:], in_=fat_d[:, None])
            eps_t = spool.tile([P, 1], f32)
            nc.gpsimd.dma_start(out=eps_t[:], in_=eps_d[:, None])
            thr_t = spool.tile([P, 1], f32)
            nc.gpsimd.dma_start(out=thr_t[:], in_=thr_d[:].to_broadcast((P, 1)))
            # noisy threshold only depends on the tiny inputs; emit it first
            # so it runs during an early VectorE idle slot, not in the tail
            noisy = spool.tile([P, 1], f32)
            nc.vector.tensor_scalar(
                out=noisy[:], in0=eps_t[:], scalar1=NOISE_SCALE, scalar2=None,
                op0=mybir.AluOpType.mult)
            nc.vector.tensor_tensor(
                out=noisy[:], in0=noisy[:], in1=thr_t[:], op=mybir.AluOpType.add)

            partial = spool.tile([P, NCHUNK], f32)
            d0 = 0
            for c, csz in enumerate(CHUNK_SIZES):
                sl = slice(d0, d0 + csz)
                d0 += csz
                xt_full = pool.tile([P, CHUNK], f32, tag="xt")
                ut_full = pool.tile([P, CHUNK], f32, tag="ut")
                wr_full = wpool.tile([3, CHUNK], bf16, tag="wr")
                xt, ut, wr = xt_full[:, :csz], ut_full[:, :csz], wr_full[:, :csz]
                nc.sync.dma_start(out=xt, in_=x_d[:, sl])
                nc.scalar.dma_start(out=ut, in_=u_d[:, sl])
                nc.gpsimd.dma_start(out=wr, in_=w3_d[:, sl])
                # broadcast w across partitions on the idle TensorE:
                # ones[3,128].T @ w3[3,N] -> psum[128,N] = w_hi+w_mid+w_lo
                for h0 in range(0, csz, PSUM_TILE):
                    hsz = min(PSUM_TILE, csz - h0)
                    wb_full = ppool.tile([P, PSUM_TILE], f32, tag="wb")
                    wb = wb_full[:, :hsz]
                    for j in range(0, hsz, MMN):
                        nc.tensor.matmul(
                            wb[:, j:j + MMN],
                            lhsT=ones[:],
                            rhs=wr[:, h0 + j:h0 + j + MMN])
                    nc.vector.tensor_tensor(
                        out=xt[:, h0:h0 + hsz],
                        in0=xt[:, h0:h0 + hsz],
                        in1=wb[:], op=mybir.AluOpType.mult)
                nc.vector.scalar_tensor_tensor(
                    out=ut, in0=ut, scalar=RELEASE_P, in1=xt,
                    op0=mybir.AluOpType.is_lt, op1=mybir.AluOpType.mult,
                    accum_out=partial[:, c:c + 1])

            ws = spool.tile([P, 1], f32)
            nc.vector.tensor_reduce(
                out=ws[:], in_=partial[:], axis=mybir.AxisListType.X,
                op=mybir.AluOpType.add)
            nc.vector.tensor_tensor(
                out=ws[:], in0=ws[:], in1=fat[:], op=mybir.AluOpType.mult)
            gate = spool.tile([P, 1], f32)
            nc.vector.tensor_tensor(
                out=gate[:], in0=ws[:], in1=noisy[:], op=mybir.AluOpType.is_gt)
            tanh_t = spool.tile([P, 1], f32)
            nc.scalar.activation(
                out=tanh_t[:], in_=ws[:], func=mybir.ActivationFunctionType.Tanh)
            nc.vector.tensor_tensor(
                out=tanh_t[:], in0=tanh_t[:], in1=gate[:], op=mybir.AluOpType.mult)
            nc.sync.dma_start(out=out_d[:, None], in_=tanh_t[:])
    nc.finalize()
    return nc


def _get_nc():
    global _NC_CACHE
    if _NC_CACHE is None:
        _NC_CACHE = _build()
    return _NC_CACHE


def _in_maps(x, w, thr, release_u, noise_eps):
    import ml_dtypes

    bf16 = ml_dtypes.bfloat16
    fat_full = (FATIGUE_DECAY ** np.arange(B, dtype=np.float64)).astype(np.float32)
    x = np.ascontiguousarray(x, dtype=np.float32)
    u = np.ascontiguousarray(release_u, dtype=np.float32)
    w = np.ascontiguousarray(w, dtype=np.float32)
    thr = np.ascontiguousarray(thr, dtype=np.float32)
    eps = np.ascontiguousarray(noise_eps, dtype=np.float32)
    # exact-to-~2^-25 split of w into three bf16 terms (summed on-chip in fp32)
    w_hi = w.astype(bf16)
    w_mid = (w - w_hi.astype(np.float32)).astype(bf16)
    w_lo = (w - w_hi.astype(np.float32) - w_mid.astype(np.float32)).astype(bf16)
    w3 = np.ascontiguousarray(np.stack([w_hi, w_mid, w_lo]))
    maps = []
    for r in range(NCORES):
        sl = slice(r * BS, (r + 1) * BS)
        maps.append({
            "x": x[sl],
            "u": u[sl],
            "w3": w3,
            "fatigue": fat_full[sl],
            "eps": eps[sl],
            "thr": thr,
        })
    return maps


def kernel(x, w, thr, release_u, noise_eps):
    from concourse import bass_utils

    nc = _get_nc()
    maps = _in_maps(x, w, thr, release_u, noise_eps)
    res = bass_utils.run_bass_kernel_spmd(nc, maps, core_ids=list(range(NCORES)))
    return np.concatenate([res.results[r]["out"] for r in range(NCORES)]).astype(np.float32)
